# revision 41
# baseline (speedup 1.0000x reference)
"""Trainium2 Bass kernel for nn_AttentionHead (B=4, S=4096, H=1024, D=64).

Reference computation (note the unusual K-first ordering):
    K = x @ Wk.T; Q = x @ Wq.T; V = x @ Wv.T            [B,S,D]
    scores[b,i,j] = (K[b,i] . Q[b,j]) / sqrt(D)         [B,S,S]
    scores[:, :, j] = -1e12 where mask[:, j] == 0
    out = softmax(scores, axis=2) @ V                   [B,S,S] @ [B,S,D]

Key structural choices:
  - Masked j-columns get softmax weight EXACTLY 0 (exp underflows), so the
    host drops them up front: the query/value axis is compacted from the
    mask (~2048 of 4096 survive) and padded to a fixed J. This halves the
    scores/exp/AV work, which dominates.
  - The t-loop is ACT(exp)-paced (~1.15us per [128,1024] tile, 34 tiles).
    The scalar engine's queue carries only the warm-up exp, three small
    early DMA triggers, and the exp stream.
  - DMA queues (sync/scalar/gpsimd) are descriptor-rate limited
    (~30-50ns/line), so all bulk uses >=1-2KB lines; the merged
    [Wq|Wv|Wk|Wk] weight tensor ships pre-shuffled as its exact SBUF
    image ([128, 2048], 4KB lines, one transfer); the tiny mask-bias
    tensor rides late; the PE-transpose identity is built on-chip from
    iota instead of being DMA'd (128 tiny lines). The critical fill set
    (weights, xk half A in proj consumption order, xq block 0) is split
    across all three queues; xq ranges stream behind, paced one block
    ahead of slot consumption; pass-B-only xk half B comes last.
  - Scores use PE row tiling (contraction D=64): the two 512-wide score
    matmuls of a slot run concurrently on row groups 0/1. This needs Q^T
    and K^T duplicated into partitions 64:128 ([Wk|Wk] stationary gives
    K^T's copy for free; Q^T via a DVE partition-shift copy).
  - AV matmuls are emitted one slot behind scores (PE queue is in-order;
    they must not head-of-line block waiting on exp). V carries a ones
    column so AV also accumulates the softmax denominator. exp outputs
    are paired two slots per SBUF tile (halves pool-semaphore traffic).
  - Two passes over the key-row halves (PSUM: double-buffered score
    tiles 4 banks + AV accumulator 2 + projection scratch 2). Pass A
    interleaves the projections between slots as HALF-units (~1us, 4
    matmuls each) on an explicit slot schedule — a long proj burst
    head-of-line starves the exp stream, while a half-unit fits inside
    the two-slot score lookahead. K^T half B's halves land late (its xk
    data is last in the DMA queues).
    Pass A's finale hides inside pass B's slot slack; 128-wide junk
    matmuls bridge DMA-wait gaps so the PE clock gate stays at 8/8.
  - Finale: out^T stays in transposed space. The softmax denominator
    row is broadcast to 64 partitions with a K=1 matmul against a ones
    stationary at partition 64, reciprocal_approx_fast + multiply on
    DVE/gpsimd, and [64, 1024] fp32 stores as out^T with 4KB lines (the
    host transposes back). At the tail the idle scalar engine takes one
    accumulator copy half.

Sharding: 8 cores = 4 batches x 2 key-row halves of 2048. Core output is
out^T [64, 2048]; host reassembles.
"""

import numpy as np

B, S, H, D = 4, 4096, 1024, 64
N_CORES = 8
SC = S // 2  # key rows (output rows) per core
HC = H // 128  # contraction chunks
J_MIN = 1024  # floor for the padded, mask-compacted query-column count
NEG = -30000.0
N_WARM = 40

_CACHE = {}


def _build(J, nfull):
    # nfull: query tiles [0, nfull) are fully kept for EVERY batch, so their
    # exp needs no mask bias (saves the ACT per-partition bias read).
    import concourse.tile as tile
    from concourse import bacc, mybir

    dt = mybir.dt
    AF = mybir.ActivationFunctionType
    JT = J // 128
    qblocks = [(c0, min(c0 + 512, J)) for c0 in range(0, J, 512)]

    nc = bacc.Bacc(
        "TRN2", target_bir_lowering=False, debug=False, num_devices=N_CORES
    )
    xtk = nc.dram_tensor("xtk", [H, SC], dt.bfloat16, kind="ExternalInput").ap()
    xtq = nc.dram_tensor("xtq", [H, J], dt.bfloat16, kind="ExternalInput").ap()
    wf = nc.dram_tensor("wf", [128, HC * 256], dt.bfloat16, kind="ExternalInput").ap()
    mb = nc.dram_tensor("mb", [128, JT], dt.float32, kind="ExternalInput").ap()
    outt = nc.dram_tensor("outt", [D, SC], dt.float32, kind="ExternalOutput").ap()

    xtk_r = xtk.rearrange("(c p) s -> p c s", p=128)
    xtq_r = xtq.rearrange("(c p) s -> p c s", p=128)

    with (
        tile.TileContext(nc) as tc,
        tc.tile_pool(name="persist", bufs=1) as persist,
        tc.tile_pool(name="ptile", bufs=4) as ptile,
    ):
        qt = persist.tile([128, J], dt.bfloat16)  # Q^T duplicated rows 0:64/64:128
        kt = persist.tile([128, SC], dt.bfloat16)  # K^T duplicated rows 0:64/64:128
        vtsb = persist.tile([128, J], dt.bfloat16)  # rows 64:128 = V^T
        vp = persist.tile([128, JT, D + 1], dt.bfloat16)
        mb_sb = persist.tile([128, JT], dt.float32)
        idb_sb = persist.tile([128, 128], dt.bfloat16)
        wsb = persist.tile([128, HC, 4 * D], dt.bfloat16)  # from wf image
        xk_sb = persist.tile([128, HC, SC], dt.bfloat16)
        xq_sb = persist.tile([128, HC, J], dt.bfloat16)
        onesb = persist.tile([D + 1, D], dt.float32)
        wtile = persist.tile([128, 128], dt.bfloat16)
        acc_sb = persist.tile([D + 1, 2, 1024], dt.float32)
        rc_sb = persist.tile([D, 2, 1024], dt.float32)
        outT = persist.tile([D, 2, 1024], dt.float32)

        nc.vector.memset(wtile[:], 0.0)
        nc.vector.memset(vp[:, :, D], 1.0)
        # ones stationary lives at partition 64: same row group as the
        # denominator row of acc_sb it broadcasts in the finale matmul
        nc.vector.memset(onesb[64:65, :], 1.0)
        # identity for the V^T->V PE transposes, built on-chip (a DMA'd
        # identity costs thousands of tiny descriptors)
        ia = persist.tile([128, 128], dt.int16)
        nc.gpsimd.iota(ia[:], [[1, 128]], base=0, channel_multiplier=-1)
        nc.gpsimd.tensor_scalar(
            idb_sb[:], ia[:], 0, None, mybir.AluOpType.is_equal
        )

        with (
            tc.tile_pool(name="psco", bufs=2, space="PSUM") as psco,
            tc.tile_pool(name="ppx", bufs=2, space="PSUM") as ppx,
            tc.tile_pool(name="pacc", bufs=1, space="PSUM") as pacc,
        ):
            dummy = persist.tile([128, 1], dt.float32)
            nc.scalar.activation(dummy[:], wtile[:, 0:1], AF.Exp)

            # --- DMA queue plans ---
            # Queue rate is descriptor-limited (~30-50ns/line): only >=1KB
            # lines for bulk, nothing tiny ahead of critical data. The
            # critical set (w4, xk half A, xq block 0) is split across all
            # three queues; mb (68B lines) rides late on scalar.
            xqranges = [(0, min(512, J))]
            c = 512
            while c < J:
                xqranges.append((c, min(c + 1024, J)))
                c += 1024

            def big_loads():
                # weight image on the scalar head (4KB lines, fast on any
                # queue) so gpsimd starts xk immediately; the projection
                # chase's first pieces land ~3us earlier
                nc.scalar.dma_start(
                    wsb[:], wf.rearrange("p (c d) -> p c d", c=HC)
                )
                # xk half A: landing order 7,0,1,2,3,4,5,6 (KORDER below)
                nc.scalar.dma_start(xk_sb[:, 7:8, 0:1024], xtk_r[:, 7:8, 0:1024])
                nc.gpsimd.dma_start(xk_sb[:, 0:2, 0:1024], xtk_r[:, 0:2, 0:1024])
                nc.gpsimd.dma_start(xk_sb[:, 2:4, 0:1024], xtk_r[:, 2:4, 0:1024])
                nc.sync.dma_start(xk_sb[:, 4:6, 0:1024], xtk_r[:, 4:6, 0:1024])
                nc.sync.dma_start(xk_sb[:, 6:7, 0:1024], xtk_r[:, 6:7, 0:1024])
                r0, r1 = xqranges[0]
                nc.gpsimd.dma_start(xq_sb[:, 0:3, r0:r1], xtq_r[:, 0:3, r0:r1])
                nc.sync.dma_start(xq_sb[:, 3:6, r0:r1], xtq_r[:, 3:6, r0:r1])
                nc.gpsimd.dma_start(xq_sb[:, 6:8, r0:r1], xtq_r[:, 6:8, r0:r1])
                for c0, c1 in xqranges[1:]:
                    nc.scalar.dma_start(
                        xq_sb[:, 0:3, c0:c1], xtq_r[:, 0:3, c0:c1]
                    )
                    nc.sync.dma_start(xq_sb[:, 3:6, c0:c1], xtq_r[:, 3:6, c0:c1])
                    nc.gpsimd.dma_start(
                        xq_sb[:, 6:8, c0:c1], xtq_r[:, 6:8, c0:c1]
                    )
                nc.scalar.dma_start(mb_sb[:], mb[:])
                nc.sync.dma_start(xk_sb[:, 0:4, 1024:2048], xtk_r[:, 0:4, 1024:2048])
                nc.gpsimd.dma_start(xk_sb[:, 4:8, 1024:2048], xtk_r[:, 4:8, 1024:2048])

            # --- PE work generators ---
            KORDER = [7, 0, 1, 2, 3, 4, 5, 6]  # xk half-A landing order

            def junk(n):  # HAM-warmth filler on the PE
                jp = psco.tile([128, 1024], dt.float32, tag="ps")
                for _ in range(n):
                    nc.tensor.matmul(
                        jp[:, 0:128], wtile[:], wtile[:], start=True, stop=True
                    )

            def proj_k_pair(sb, order, sprinkle=False):
                # kt blocks [1024*sb, 1024*sb+1024), per-hc interleaved
                c0 = 1024 * sb
                psL = ppx.tile([128, 512], dt.float32, tag="px")
                psR = ppx.tile([128, 512], dt.float32, tag="px")
                for n, hc in enumerate(order):
                    nc.tensor.matmul(
                        psL[:],
                        wsb[:, hc, 128:256],
                        xk_sb[:, hc, c0 : c0 + 512],
                        start=(n == 0),
                        stop=(n == HC - 1),
                    )
                    nc.tensor.matmul(
                        psR[:],
                        wsb[:, hc, 128:256],
                        xk_sb[:, hc, c0 + 512 : c0 + 1024],
                        start=(n == 0),
                        stop=(n == HC - 1),
                    )
                    if sprinkle and n in (0, 2):
                        junk(8)  # bridge DMA-chase gaps, keep HAM warm
                nc.vector.tensor_copy(kt[:, c0 : c0 + 512], psL[:])
                nc.vector.tensor_copy(kt[:, c0 + 512 : c0 + 1024], psR[:])

            def proj_qv(bi):  # [Q^T; V^T] for one query-column block
                c0, c1 = qblocks[bi]
                ps = ppx.tile([128, c1 - c0], dt.float32, tag="px")
                for hc in range(HC):
                    nc.tensor.matmul(
                        ps[:],
                        wsb[:, hc, 0:128],
                        xq_sb[:, hc, c0:c1],
                        start=(hc == 0),
                        stop=(hc == HC - 1),
                    )
                nc.vector.tensor_copy(qt[0:64, c0:c1], ps[0:64, :])
                nc.vector.tensor_copy(vtsb[64:128, c0:c1], ps[64:128, :])
                # duplicate Q^T into partitions 64:128 for row-tiled scores
                nc.vector.tensor_copy(qt[64:128, c0:c1], ps[0:64, :])

            def vt_block(st0, st1):  # V^T -> V via PE transpose
                for st in range(st0, st1):
                    pvt = ppx.tile([128, D], dt.bfloat16, tag="px")
                    nc.tensor.transpose(
                        pvt[:],
                        vtsb[64:128, 128 * st : 128 * (st + 1)],
                        idb_sb[64:128, 64:128],
                    )
                    nc.vector.tensor_copy(vp[:, st, 0:D], pvt[:])

            # --- t-loop slot machinery: AV deferred one slot behind ---
            pending = []
            ptpair = [None]
            ptslot = [0]

            def flush_av():
                if not pending:
                    return
                pt, t, acc = pending.pop()
                for nb in range(2):
                    nc.tensor.matmul(
                        acc[:, 512 * nb : 512 * (nb + 1)],
                        vp[:, t, :],
                        pt[:, 512 * nb : 512 * (nb + 1)],
                        start=(t == 0),
                        stop=(t == JT - 1),
                    )

            def t_slot(t, acc, ih):
                ps = psco.tile([128, 1024], dt.float32, tag="ps")
                # row-tiled pair: groups 0/1 run concurrently (contraction 64)
                nc.tensor.matmul(
                    ps[:, 0:512],
                    qt[0:64, 128 * t : 128 * (t + 1)],
                    kt[0:64, 1024 * ih : 1024 * ih + 512],
                    start=True,
                    stop=True,
                )
                nc.tensor.matmul(
                    ps[:, 512:1024],
                    qt[64:128, 128 * t : 128 * (t + 1)],
                    kt[64:128, 1024 * ih + 512 : 1024 * ih + 1024],
                    start=True,
                    stop=True,
                )
                flush_av()
                # pt tiles are paired (2 slots per tile): halves the pool
                # allocation count and its semaphore traffic
                if ptslot[0] % 2 == 0:
                    ptpair[0] = ptile.tile(
                        [128, 2048], dt.bfloat16, tag="pt", name="ptp"
                    )
                pt = ptpair[0][:, 1024 * (ptslot[0] % 2) :][:, 0:1024]
                ptslot[0] += 1
                if t < nfull:
                    nc.scalar.activation(pt, ps[:], AF.Exp, scale=0.125)
                else:
                    nc.scalar.activation(
                        pt, ps[:], AF.Exp, bias=mb_sb[:, t : t + 1], scale=0.125
                    )
                pending.append((pt, t, acc))

            def acc_to_sb(acc, ih, tail=False):
                # copy PSUM acc to SBUF, freeing pacc; at the kernel tail
                # the scalar engine is idle and takes one half
                nc.vector.tensor_copy(acc_sb[:, ih, 0:512], acc[:, 0:512])
                if tail:
                    nc.scalar.copy(acc_sb[:, ih, 512:1024], acc[:, 512:1024])
                else:
                    nc.vector.tensor_copy(acc_sb[:, ih, 512:1024], acc[:, 512:1024])

            def finale_steps(ih, tail=False):
                # divide in transposed space; store out^T with 4KB lines
                steps = []
                for half in range(2):
                    c0, c1 = 512 * half, 512 * (half + 1)

                    def bcast(h0=c0, h1=c1, i=ih):
                        # denominator row (partition 64) broadcast to 64
                        # partitions via K=1 matmul in row group 2
                        pd = ppx.tile([128, 512], dt.float32, tag="px")
                        nc.tensor.matmul(
                            pd[0:64, :],
                            onesb[64:65, :],
                            acc_sb[64:65, i, h0:h1],
                            start=True,
                            stop=True,
                        )
                        nc.vector.reciprocal_approx_fast(
                            rc_sb[:, i, h0:h1], pd[0:64, :]
                        )

                    def mult(h0=c0, h1=c1, i=ih):
                        eng = nc.vector if tail else nc.gpsimd
                        eng.tensor_mul(
                            outT[:, i, h0:h1],
                            acc_sb[0:64, i, h0:h1],
                            rc_sb[:, i, h0:h1],
                        )

                    def store(h0=c0, h1=c1, i=ih):
                        nc.sync.dma_start(
                            outt[:, 1024 * i + h0 : 1024 * i + h1],
                            outT[:, i, h0:h1],
                        )

                    steps.append(bcast)
                    steps.append(mult)
                    steps.append(store)
                return steps

            # ---- pass A (i-half 0) interleaved with the projections ----
            big_loads()
            accA = pacc.tile([D + 1, 1024], dt.float32, tag="acc")
            tA = lambda t: t_slot(t, accA, 0)
            # PE warmup (128-wide junk) while the first slices stream in
            junk(N_WARM)
            proj_k_pair(0, KORDER, sprinkle=True)
            proj_qv(0)
            vt_block(0, qblocks[0][1] // 128)

            # Remaining projections run as HALF-units (~1us each) on an
            # explicit slot schedule: the two-slot exp lookahead absorbs
            # each half, where full 2-3us units starved the exp stream.
            # The PSUM accumulator is held open across the gap (PSUM
            # accumulation tolerates interleaved matmuls to other banks).
            def qv_half_unit(bi):
                c0, c1 = qblocks[bi]
                cell = [None]

                def h1():
                    cell[0] = ppx.tile(
                        [128, c1 - c0], dt.float32, tag="px", name="psq"
                    )
                    for hc in range(4):
                        nc.tensor.matmul(
                            cell[0][:],
                            wsb[:, hc, 0:128],
                            xq_sb[:, hc, c0:c1],
                            start=(hc == 0),
                            stop=False,
                        )

                def h2():
                    for hc in range(4, HC):
                        nc.tensor.matmul(
                            cell[0][:],
                            wsb[:, hc, 0:128],
                            xq_sb[:, hc, c0:c1],
                            start=False,
                            stop=(hc == HC - 1),
                        )

                def fin():
                    ps = cell[0]
                    nc.vector.tensor_copy(qt[0:64, c0:c1], ps[0:64, :])
                    nc.vector.tensor_copy(vtsb[64:128, c0:c1], ps[64:128, :])
                    nc.vector.tensor_copy(qt[64:128, c0:c1], ps[0:64, :])
                    vt_block(c0 // 128, c1 // 128)

                return h1, h2, fin

            def k_half_unit(c0):
                cell = [None]

                def h1():
                    cell[0] = ppx.tile(
                        [128, 512], dt.float32, tag="px", name="psk"
                    )
                    for hc in range(4):
                        nc.tensor.matmul(
                            cell[0][:],
                            wsb[:, hc, 128:256],
                            xk_sb[:, hc, c0 : c0 + 512],
                            start=(hc == 0),
                            stop=False,
                        )

                def h2():
                    for hc in range(4, HC):
                        nc.tensor.matmul(
                            cell[0][:],
                            wsb[:, hc, 128:256],
                            xk_sb[:, hc, c0 : c0 + 512],
                            start=False,
                            stop=(hc == HC - 1),
                        )
                    nc.vector.tensor_copy(kt[:, c0 : c0 + 512], cell[0][:])

                return h1, h2

            sched = {}
            for b in range(1, len(qblocks)):
                h1, h2, fin = qv_half_unit(b)
                s = 3 * (b - 1)
                sched.setdefault(s, []).append(h1)
                sched.setdefault(s + 1, []).append(h2)
                sched.setdefault(s + 2, []).append(fin)
            kbase = JT - 5
            for ki, c0 in enumerate((1024, 1536)):
                h1, h2 = k_half_unit(c0)
                sched.setdefault(kbase + 2 * ki, []).append(h1)
                sched.setdefault(kbase + 2 * ki + 1, []).append(h2)
            for t in range(JT):
                tA(t)
                for op in sched.get(t, []):
                    op()
            flush_av()
            acc_to_sb(accA, 0)

            # ---- pass B (i-half 1), finale A interleaved into its slack ----
            accB = pacc.tile([D + 1, 1024], dt.float32, tag="acc")
            finA = finale_steps(0)
            for t in range(JT):
                t_slot(t, accB, 1)
                if finA and t >= 3 and t % 2 == 1:
                    finA.pop(0)()
            while finA:
                finA.pop(0)()
            flush_av()
            acc_to_sb(accB, 1, tail=True)
            for step in finale_steps(1, tail=True):
                step()

    nc.compile()
    return nc


def _in_maps(x, mask, Wk, Wq, Wv):
    import ml_dtypes

    bf16 = ml_dtypes.bfloat16
    w4 = np.concatenate([Wq.T, Wv.T, Wk.T, Wk.T], axis=1).astype(bf16)
    # pre-shuffled SBUF image: [128, HC*256], 4KB contiguous per partition
    wf = np.ascontiguousarray(
        w4.reshape(HC, 128, 4 * D).transpose(1, 0, 2).reshape(128, HC * 4 * D)
    )
    nk = [int((mask[b] != 0).sum()) for b in range(B)]
    J = max(J_MIN, -(-max(nk) // 128) * 128)
    nfull = min(nk) // 128
    JT = J // 128
    xtq_b, mb_b = [], []
    for b in range(B):
        idx = np.flatnonzero(mask[b] != 0)
        xt = np.zeros((H, J), dtype=bf16)
        xt[:, : len(idx)] = x[b].T[:, idx].astype(bf16)
        xtq_b.append(xt)
        mbv = np.full(J, np.float32(NEG), dtype=np.float32)
        mbv[: len(idx)] = 0.0
        mb_b.append(np.ascontiguousarray(mbv.reshape(JT, 128).T))
    maps = []
    for c in range(N_CORES):
        b, half = c // 2, c % 2
        xtk = np.ascontiguousarray(x[b, half * SC : (half + 1) * SC].T.astype(bf16))
        maps.append(
            {
                "xtk": xtk,
                "xtq": xtq_b[b],
                "wf": wf,
                "mb": mb_b[b],
            }
        )
    return maps, (J, nfull)


def kernel(x, mask, Wk, Wq, Wv):
    from concourse.bass_utils import run_bass_kernel_spmd

    maps, key = _in_maps(x, mask, Wk, Wq, Wv)
    if key not in _CACHE:
        _CACHE[key] = _build(*key)
    nc = _CACHE[key]
    br = run_bass_kernel_spmd(nc, maps, list(range(N_CORES)))
    out = np.empty((B, S, D), dtype=np.float32)
    for c in range(N_CORES):
        b, half = c // 2, c % 2
        out[b, half * SC : (half + 1) * SC, :] = br.results[c]["outt"].T
    return out
